# revision 36
# baseline (speedup 1.0000x reference)
"""Trainium2 Bass kernel for nn_BindingSiteGraphSAGE (3-layer GraphSAGE + MLP head).

Self-contained: takes the full inputs, shards destination nodes across the
8 NeuronCores, runs a single SPMD Bass program (edge aggregation via
indirect-DMA gathers + weighted-onehot PSUM matmuls, with per-superblock
interleaved dense phases and per-superblock AllGather collectives between
layers), and returns the full [50000, 2] float32 output.

Optimizations over the first working version:
- all dense matmuls run in bf16 (fp32 costs 4 cycles/row on the PE).
- onehot builds use a bf16 iota so the DVE runs in its 2x perf mode.
- gather chunks are one (block, half) group each with num_idxs trimmed to
  the real edge count, so tile-padding slots are never fetched from HBM
  (the onehot masks the stale SBUF contents).
- leaky ReLU is a single Prelu activation (alpha=0.15).
- the L2/L3 self terms stay resident in SBUF (no DRAM roundtrip).
- gather tables (x copy, t2, t3) share one superblock-major row layout
  ("pid" space): each superblock's AllGather output is contiguous, so the
  AllGathers run per superblock as soon as that superblock's table rows are
  computed, overlapping the remaining compute; only the last small chunk's
  latency is exposed. One gather-index table serves all three layers.

Host preprocessing only reorders/pads the edge list and ships structural
metadata (indices, degree weights, schedules, a row-permuted copy of x) —
no feature math on host.
"""
import sys
for _p in ("/opt/trn_rl_repo",):
    if _p not in sys.path:
        sys.path.insert(0, _p)
import numpy as np
import concourse.bass as bass
import concourse.bacc as bacc
import concourse.tile as tile
import concourse.mybir as mybir
from concourse.bass_utils import run_bass_kernel_spmd

"""Host-side graph structure preprocessing.

Partitions dst nodes across cores, builds a uniform (core-independent)
tile schedule for edge aggregation, and per-core gather/onehot buffers.

Layout conventions:
- dst slice per core: cfg['slice'] real nodes, padded to cfg['slice_pad'].
- dst blocks of W=128 local dsts; superblocks per cfg['sb_sizes'].
- gather-table rows live in "pid" space, superblock-major:
  pid = 8192*sb + core*sbc(sb) + (local_row - 1024*sb); superblocks 0-3
  occupy pids [0, 32768) ("lo"), 4-6 occupy [32768, 50176) ("hi"), so
  int16 gather indices are pid or pid - 32768.
- edges split lo/hi by the src's superblock (local_row < 4096).
- per (sb, half, block): fixed tile count = max over cores (>=1 for lo).
- gather idx i -> partition i%16, free i//16 (replicated x8 over partitions).
- seg/wgt per edge-slot: partition i%128, free tile index (seg 128 = pad).
"""

W = 128


def make_cfg(n_cores, n_nodes, slice_, slice_pad, sb_sizes):
    assert slice_pad % W == 0 and sum(sb_sizes) == slice_pad // W
    assert sb_sizes == [8, 8, 8, 8, 8, 8, 1], "pid map hardcodes this layout"
    return dict(n_cores=n_cores, n_nodes=n_nodes, slice=slice_,
                slice_pad=slice_pad, sb_sizes=sb_sizes,
                half_r=4096, hi_base=32768, n_lo_sb=4,
                pid_n=n_cores * slice_pad)


CFG_FULL = make_cfg(8, 50000, 6250, 6272, [8, 8, 8, 8, 8, 8, 1])

# gather chunks are (block, half) groups, merged up to this many tiles while
# still ending on a group boundary (the trailing group's padding is skipped
# via num_idxs). hi-half groups are smaller (35% of edges) which makes their
# gathers Pool-desc-gen-bound, so they merge in pairs.
MAX_CHUNK_TILES = 24
GROUPS_PER_CHUNK = {0: 1, 1: 2}


def pid_of(src, cfg):
    """node id -> row in the superblock-major gather tables (vectorized)."""
    src = np.asarray(src, dtype=np.int64)
    SL, C = cfg["slice"], cfg["n_cores"]
    r = src % SL
    c = src // SL
    s = r // 1024                      # 6144..6249 -> 6 (last, short sb)
    sbc = np.where(s < 6, 1024, 128)
    return 8192 * s + c * sbc + (r - 1024 * s)


def build_structure(edge_index, cfg):
    src = np.asarray(edge_index[0], dtype=np.int64)
    dst = np.asarray(edge_index[1], dtype=np.int64)
    C, SL, SP = cfg["n_cores"], cfg["slice"], cfg["slice_pad"]
    HALF_R, HI_BASE = cfg["half_r"], cfg["hi_base"]

    percore_edges = []
    for c in range(C):
        m = (dst >= c * SL) & (dst < (c + 1) * SL)
        es, ed = src[m], dst[m] - c * SL
        cnt = np.bincount(ed, minlength=SP).astype(np.float32)
        wall = 1.0 / np.maximum(cnt, 1.0)
        percore_edges.append((es, ed, wall))

    sched = []
    percore = [dict(idx=[], seg=[], wgt=[]) for _ in range(C)]
    b0 = 0
    for sb_sz in cfg["sb_sizes"]:
        blocks = list(range(b0, b0 + sb_sz))
        ntiles = {0: [], 1: []}
        maxe = {0: [], 1: []}
        elists = {0: [], 1: []}
        for h in (0, 1):
            for b in blocks:
                per_core_list = []
                mx = 1 if h == 0 else 0
                me = 0
                for c in range(C):
                    es, ed, wall = percore_edges[c]
                    hlo = (es % SL) < HALF_R
                    m = (ed >= b * W) & (ed < (b + 1) * W) & \
                        (hlo if h == 0 else ~hlo)
                    per_core_list.append((es[m], ed[m]))
                    mx = max(mx, (len(es[m]) + W - 1) // W)
                    me = max(me, len(es[m]))
                ntiles[h].append(mx)
                maxe[h].append(me)
                elists[h].append(per_core_list)

        tiles = []   # (half, b_local)
        for h in (0, 1):
            for bl in range(sb_sz):
                tiles += [(h, bl)] * ntiles[h][bl]

        for c in range(C):
            for h in (0, 1):
                for bl in range(sb_sz):
                    es, ed = elists[h][bl][c]
                    wall = percore_edges[c][2]
                    nslot = ntiles[h][bl] * W
                    ix = np.zeros(nslot, np.int16)
                    sg = np.full(nslot, W, np.float32)
                    wg = np.zeros(nslot, np.float32)
                    ne = len(es)
                    assert ne <= nslot
                    pv = pid_of(es, cfg)
                    if h == 1:
                        pv = pv - HI_BASE
                    assert ne == 0 or (pv.min() >= 0 and pv.max() < 32768)
                    ix[:ne] = pv.astype(np.int16)
                    sg[:ne] = (ed - blocks[bl] * W).astype(np.float32)
                    wg[:ne] = wall[ed]
                    percore[c]["idx"].append(ix)
                    percore[c]["seg"].append(sg)
                    percore[c]["wgt"].append(wg)

        # chunks: merge up to GROUPS_PER_CHUNK (block, half) groups while
        # <= MAX_CHUNK_TILES tiles; num_idxs trimmed to skip the trailing
        # group's padding slots.
        chunks = []
        t = 0
        for h in (0, 1):
            run = []  # (nt, maxE) for groups pending merge
            for bl in range(sb_sz):
                nt_g, me_g = ntiles[h][bl], maxe[h][bl]
                if run and (len(run) >= GROUPS_PER_CHUNK[h] or
                            sum(r[0] for r in run) + nt_g > MAX_CHUNK_TILES):
                    tot = sum(r[0] for r in run)
                    lead = tot - run[-1][0]
                    nidx = lead * W + max(run[-1][1], 1)
                    nidx = min(tot * W, 16 * ((nidx + 15) // 16))
                    nidx = max(nidx, (tot - 1) * W + 16)
                    chunks.append((h, t, t + tot, nidx))
                    t += tot
                    run = []
                run.append((nt_g, me_g))
            if run:
                tot = sum(r[0] for r in run)
                lead = tot - run[-1][0]
                nidx = lead * W + max(run[-1][1], 1)
                nidx = min(tot * W, 16 * ((nidx + 15) // 16))
                nidx = max(nidx, (tot - 1) * W + 16)
                chunks.append((h, t, t + tot, nidx))
                t += tot
        assert t == len(tiles)
        sched.append(dict(ntiles=ntiles, tiles=tiles, chunks=chunks,
                          blocks=blocks, sb_sz=sb_sz))
        b0 += sb_sz

    for c in range(C):
        for k in ("idx", "seg", "wgt"):
            percore[c][k] = np.concatenate(percore[c][k])
    return sched, percore


def pack_gather_idx(flat_idx):
    """[n] int16 -> [128, n/16] buffer (16-partition wrap, replicated x8)."""
    n = len(flat_idx)
    assert n % 16 == 0
    b = flat_idx.reshape(n // 16, 16).T
    return np.tile(b, (8, 1)).copy()


def pack_tilewise(flat, ntiles):
    """[n=ntiles*128] -> [128, ntiles] (edge i -> part i%128, free i//128)."""
    return flat.reshape(ntiles, W).T.copy()


F32 = mybir.dt.float32
BF16 = mybir.dt.bfloat16
AF = mybir.ActivationFunctionType
ALU = mybir.AluOpType
W = 128
SLOPE = 0.15

# all small constants ship as two packed tensors (one bf16, one f32 that also
# carries seg/wgt) so startup is 2 big DMAs instead of ~24 serialized HWDGE ops
WPACK_BF = ([("iota", 128, 128), ("W1l", 128, 512), ("W1r", 128, 512)]
            + [(f"W2l{k}", 128, 256) for k in range(4)]
            + [(f"W2r{k}", 128, 256) for k in range(4)]
            + [(f"W3l{k}", 128, 64) for k in range(2)]
            + [(f"W3r{k}", 128, 64) for k in range(2)]
            + [("Wp", 64, 32), ("Wf1", 32, 32), ("Wf2", 32, 2),
               ("ones", 1, 512), ("bf2r", 1, 2)])
WPACK_F32 = [("b1c", 128, 4), ("b2c", 128, 2), ("b3c", 64, 1),
             ("bpc", 32, 1), ("bf1c", 32, 1)]


def pack_offsets(spec):
    offs, o = {}, 0
    for name, part, cols in spec:
        offs[name] = (o, part, cols)
        o += cols
    return offs, o


BF_OFFS, BF_COLS = pack_offsets(WPACK_BF)
F32_OFFS, F32_COLS = pack_offsets(WPACK_F32)


def ceil_div(a, b):
    return (a + b - 1) // b


def build_kernel(cfg, sched, timing_mode=False):
    """cfg: dict(n_cores, n_nodes, slice, slice_pad, sb_sizes, ...)
    sched: from build_structure (list of superblock dicts)."""
    C = cfg["n_cores"]
    SP = cfg["slice_pad"]
    HI_BASE = cfg["hi_base"]
    N_LO_SB = cfg["n_lo_sb"]
    PIDN = cfg["pid_n"]
    NTT = sum(len(s["tiles"]) for s in sched)      # total tiles
    NID = NTT * W                                   # total edge slots
    MAXSBC = max(s["sb_sz"] for s in sched) * W
    MAXNT = max(t1 - t0 for s in sched for (_, t0, t1, _) in s["chunks"])
    sb_rows = [C * s["sb_sz"] * W for s in sched]   # pid rows per superblock
    sb_row0 = np.cumsum([0] + sb_rows).tolist()
    HI_ROWS = PIDN - HI_BASE

    def sb_chunks(sbc):
        return [(i * 512, min(512, sbc - i * 512)) for i in range(ceil_div(sbc, 512))]

    nc = bacc.Bacc("TRN2", target_bir_lowering=False, debug=False,
                   enable_asserts=True, num_devices=(1 if timing_mode else C))

    # ---------------- DRAM I/O ----------------
    xpid_d = nc.dram_tensor("xpid", [PIDN, 128], BF16, kind="ExternalInput")
    xT_d = nc.dram_tensor("xT", [128, SP], BF16, kind="ExternalInput")
    idx_d = nc.dram_tensor("idx", [128, NID // 16], mybir.dt.int16, kind="ExternalInput")
    wb_d = nc.dram_tensor("wb", [128, BF_COLS], BF16, kind="ExternalInput")
    wf_d = nc.dram_tensor("wf", [128, 2 * NTT + F32_COLS], F32, kind="ExternalInput")
    out_d = nc.dram_tensor("out", [SP, 2], F32, kind="ExternalOutput")

    with tile.TileContext(nc) as tc:
        with (
            tc.tile_pool(name="const", bufs=1) as cp,
            tc.tile_pool(name="big", bufs=1) as bp,       # long-lived buffers
            tc.tile_pool(name="gath", bufs=4) as gp,      # gather chunks
            tc.tile_pool(name="work", bufs=3) as wp,      # onehot / drains / stages
            tc.tile_pool(name="psA", bufs=1, space="PSUM") as psA,   # agg psums
            tc.tile_pool(name="psB", bufs=2, space="PSUM") as psB,   # dense psums
            tc.tile_pool(name="dram", bufs=1, space="DRAM") as dp,
        ):
            # ---------------- constants ----------------
            wf_t = cp.tile([128, 2 * NTT + F32_COLS], F32, name="wf_t")
            nc.sync.dma_start(wf_t[:], wf_d.ap())
            wb_t = cp.tile([128, BF_COLS], BF16, name="wb_t")
            nc.sync.dma_start(wb_t[:], wb_d.ap())

            def w(name, c0=None, c1=None):
                o, p, c = BF_OFFS[name]
                if c0 is None:
                    c0, c1 = 0, c
                return wb_t[0:p, o + c0:o + c1]

            def wf(name, c0=None, c1=None):
                o, p, c = F32_OFFS[name]
                if c0 is None:
                    c0, c1 = 0, c
                return wf_t[0:p, 2 * NTT + o + c0:2 * NTT + o + c1]

            # persistent self terms (SBUF-resident)
            selfb2 = [cp.tile([128, SP], BF16, name=f"selfb2_{m}") for m in range(2)]
            selfb3 = cp.tile([64, SP], BF16)

            # DRAM scratch: allgathered tables, split lo/hi so gathers from
            # the lo pids only depend on the lo superblocks' collectives
            t2_lo = dp.tile([HI_BASE, 256], BF16)
            t2_hi = dp.tile([HI_ROWS, 256], BF16)
            t3_lo = dp.tile([HI_BASE, 128], BF16)
            t3_hi = dp.tile([HI_ROWS, 128], BF16)
            t2_slice = dp.tile([SP, 256], BF16)
            t3_slice = dp.tile([SP, 128], BF16)

            def emit_ag(si, slice_t, lo_t, hi_t):
                if timing_mode:
                    return
                col0 = sched[si]["blocks"][0] * W
                sbc = sched[si]["sb_sz"] * W
                r0 = sb_row0[si]
                dst_t, doff = (lo_t, 0) if si < N_LO_SB else (hi_t, HI_BASE)
                nc.gpsimd.collective_compute(
                    "AllGather", ALU.bypass, replica_groups=[list(range(C))],
                    ins=[slice_t[col0:col0 + sbc, :].opt()],
                    outs=[dst_t[r0 - doff:r0 - doff + C * sbc, :].opt()])

            # prime the gather-pool buffers so slots skipped by trimmed
            # num_idxs never expose uninitialized SBUF (NaN x 0 = NaN in PE)
            for pi in range(4):
                gz = gp.tile([128, MAXNT * 256], BF16, name=f"gz{pi}", tag="g")
                nc.vector.memset(gz[:], 0.0)

            def leaky(dst_ap, src_ap, bias=0.0):
                # Prelu honors alpha on HW (Lrelu ignores it: fixed slope)
                nc.scalar.activation(dst_ap, src_ap, AF.Prelu, bias=bias,
                                     alpha=SLOPE)

            # =========== generic aggregation phase ===========
            def agg_phase(d, row_elems, lo_ap, hi_ap, idx_t, drain_fn,
                          after_sb, tag):
                """d: used dims; row_elems: table row width.
                drain_fn(si, mp, ps_ap, col0, ncols): consume psum [dmp, ncols]
                holding agg_T rows [mp*128, ...) cols [col0, col0+ncols).
                after_sb(si, col0, sbc): emit dependent work for the sb."""
                n_mp = ceil_div(d, 128)
                dmp = min(d, 128)
                toff = 0
                for si, s in enumerate(sched):
                    sbc = s["sb_sz"] * W
                    tiles = s["tiles"]
                    bank_of = lambda bl: (bl * W) // 512
                    first, last = {}, {}
                    for i, (h, bl) in enumerate(tiles):
                        bk = bank_of(bl)
                        first.setdefault(bk, i)
                        last[bk] = i
                    ps = [psA.tile([dmp, sbc], F32, name=f"ps{tag}{mp}",
                                   tag=f"aggps{mp}", bufs=(2 if mp == 0 else 1))
                          for mp in range(n_mp)]
                    for (h, t0, t1, nidx) in s["chunks"]:
                        nt = t1 - t0
                        g = gp.tile([128, nt, row_elems], BF16, name=f"g{tag}", tag="g")
                        base = lo_ap if h == 0 else hi_ap
                        nc.gpsimd.dma_gather(
                            g[:, :nt, :], base,
                            idx_t[:, (toff + t0) * 8:(toff + t1) * 8],
                            nidx, nidx, elem_size=row_elems, single_packet=False)
                        for t in range(t0, t1):
                            h_, bl = tiles[t]
                            bk = bank_of(bl)
                            oh = wp.tile([128, W], BF16, name=f"oh{tag}", tag="oh", bufs=16)
                            gt = toff + t
                            nc.vector.tensor_scalar(
                                oh[:], w("iota"), wf_t[:, gt:gt + 1],
                                wf_t[:, NTT + gt:NTT + gt + 1],
                                ALU.is_equal, ALU.mult)
                            for mp in range(n_mp):
                                nc.tensor.matmul(
                                    ps[mp][:, bl * W:(bl + 1) * W],
                                    g[:, t - t0, mp * dmp:(mp + 1) * dmp],
                                    oh[:],
                                    start=(first[bk] == t), stop=(last[bk] == t))
                    col0 = s["blocks"][0] * W
                    # drain the single-buffered mps first: the next
                    # superblock's matmuls wait on their psum buffers
                    for mp in reversed(range(n_mp)):
                        cc = 0
                        while cc < sbc:
                            n = min(512, sbc - cc)
                            drain_fn(si, mp, ps[mp][:, cc:cc + n], col0 + cc, n)
                            cc += n
                    if after_sb is not None:
                        after_sb(si, col0, sbc)
                    toff += len(tiles)

            # =========== L1: agg + interleaved dense ===========
            # only the first superblock's indices load upfront; the rest (and
            # most of xT) load behind the first superblock's gathers so the
            # DMA engines start useful work immediately
            idx_t = bp.tile([128, NID // 16], mybir.dt.int16, name="idx_t", tag="big1")
            NT0 = len(sched[0]["tiles"])
            SBC0 = sched[0]["sb_sz"] * W
            nc.sync.dma_start(idx_t[:, :NT0 * 8], idx_d.ap()[:, :NT0 * 8])
            xT_t = bp.tile([128, SP], BF16, name="xT_t", tag="big3")
            nc.sync.dma_start(xT_t[:, :SBC0], xT_d.ap()[:, :SBC0])

            agg1_sb = {}

            def drain1(si, mp, ps_ap, c0, n):
                if si not in agg1_sb:
                    agg1_sb[si] = (bp.tile([128, MAXSBC], BF16, name=f"agg1_{si}",
                                           tag="agg1", bufs=3),
                                   sched[si]["blocks"][0] * W)
                t_, col0 = agg1_sb[si]
                nc.scalar.activation(t_[:, c0 - col0:c0 - col0 + n], ps_ap, AF.Copy)

            def l1_dense(si, col0, sbc):
                if si == 0:
                    nc.sync.dma_start(idx_t[:, NT0 * 8:], idx_d.ap()[:, NT0 * 8:])
                    nc.sync.dma_start(xT_t[:, SBC0:], xT_d.ap()[:, SBC0:])
                a1, _ = agg1_sb.pop(si)
                for (off, n) in sb_chunks(sbc):
                    c0 = col0 + off
                    h1 = [wp.tile([128, 512], BF16, name=f"h1_{m}", tag=f"h1_{m}",
                                  bufs=2) for m in range(4)]
                    for m in range(4):
                        ph = psB.tile([128, 512], F32, name="ph1", tag="dense")
                        nc.tensor.matmul(ph[:, :n], w("W1l", m * 128, (m + 1) * 128),
                                         a1[:, off:off + n], start=True, stop=False)
                        nc.tensor.matmul(ph[:, :n], w("W1r", m * 128, (m + 1) * 128),
                                         xT_t[:, c0:c0 + n], start=False, stop=True)
                        leaky(h1[m][:, :n], ph[:, :n], bias=wf("b1c", m, m + 1))
                    for sub in range(n // 128):
                        pt = psB.tile([128, 256], F32, name="pt2", tag="dense")
                        for k in range(4):
                            nc.tensor.matmul(pt[:], h1[k][:, sub * 128:(sub + 1) * 128],
                                             w(f"W2l{k}"),
                                             start=(k == 0), stop=(k == 3))
                        st = wp.tile([128, 256], BF16, name="st2", tag="st2")
                        nc.scalar.activation(st[:], pt[:], AF.Copy)
                        r0 = c0 + sub * 128
                        nc.sync.dma_start(t2_slice[r0:r0 + 128, :], st[:])
                    for m in range(2):
                        pss = psB.tile([128, 512], F32, name="ps2", tag="dense")
                        for k in range(4):
                            nc.tensor.matmul(pss[:, :n],
                                             w(f"W2r{k}", m * 128, (m + 1) * 128),
                                             h1[k][:, :n], start=(k == 0), stop=(k == 3))
                        nc.scalar.activation(selfb2[m][:, c0:c0 + n], pss[:, :n],
                                             AF.Identity, bias=wf("b2c", m, m + 1))
                emit_ag(si, t2_slice, t2_lo, t2_hi)

            agg_phase(128, 128, xpid_d.ap(), xpid_d.ap()[HI_BASE:, :],
                      idx_t, drain1, l1_dense, "1")

            # =========== L2 ===========
            h2_sb = {}

            def drain2(si, mp, ps_ap, c0, n):
                key = (si, mp)
                if key not in h2_sb:
                    h2_sb[key] = (bp.tile([128, MAXSBC], BF16, name=f"h2_{si}_{mp}",
                                          tag=f"h2m{mp}", bufs=2),
                                  sched[si]["blocks"][0] * W)
                t_, col0 = h2_sb[key]
                pre = wp.tile([128, 512], BF16, name="pre2", tag="pre")
                nc.vector.tensor_tensor(pre[:, :n], ps_ap,
                                        selfb2[mp][:, c0:c0 + n], ALU.add)
                leaky(t_[:, c0 - col0:c0 - col0 + n], pre[:, :n])

            def l2_dense(si, col0, sbc):
                h2m = [h2_sb.pop((si, mp))[0] for mp in range(2)]
                for sub in range(sbc // 128):
                    pt = psB.tile([128, 64], F32, name="pt3", tag="dense")
                    s0 = col0 + sub * 128
                    for k in range(2):
                        nc.tensor.matmul(pt[:], h2m[k][:, sub * 128:(sub + 1) * 128],
                                         w(f"W3l{k}"),
                                         start=(k == 0), stop=(k == 1))
                    st = wp.tile([128, 64], BF16, name="st3", tag="st3")
                    nc.scalar.activation(st[:], pt[:], AF.Copy)
                    nc.sync.dma_start(t3_slice[s0:s0 + 128, :64], st[:])
                for (off, n) in sb_chunks(sbc):
                    c0 = col0 + off
                    pss = psB.tile([64, 512], F32, name="ps3", tag="dense")
                    for k in range(2):
                        nc.tensor.matmul(pss[:, :n], w(f"W3r{k}"),
                                         h2m[k][:, off:off + n],
                                         start=(k == 0), stop=(k == 1))
                    nc.scalar.activation(selfb3[:, c0:c0 + n], pss[:, :n],
                                         AF.Identity, bias=wf("b3c"))
                emit_ag(si, t3_slice, t3_lo, t3_hi)

            agg_phase(256, 256, t2_lo[:], t2_hi[:], idx_t,
                      drain2, l2_dense, "2")

            # =========== L3 + head ===========
            h3_sb = {}

            def drain3(si, mp, ps_ap, c0, n):
                if si not in h3_sb:
                    h3_sb[si] = (bp.tile([64, MAXSBC], BF16, name=f"h3_{si}",
                                         tag="h3", bufs=2),
                                 sched[si]["blocks"][0] * W)
                t_, col0 = h3_sb[si]
                pre = wp.tile([64, 512], BF16, name="pre3", tag="pre")
                nc.vector.tensor_tensor(pre[:, :n], ps_ap,
                                        selfb3[:, c0:c0 + n], ALU.add)
                leaky(t_[:, c0 - col0:c0 - col0 + n], pre[:, :n])

            def head(si, col0, sbc):
                h3t, _ = h3_sb.pop(si)
                for (off, n) in sb_chunks(sbc):
                    c0 = col0 + off
                    pp = psB.tile([32, 512], F32, name="pp", tag="dense")
                    nc.tensor.matmul(pp[:, :n], w("Wp"), h3t[:, off:off + n],
                                     start=True, stop=True)
                    p_t = wp.tile([32, 512], BF16, name="p", tag="p", bufs=2)
                    nc.scalar.activation(p_t[:, :n], pp[:, :n], AF.Identity,
                                         bias=wf("bpc"))

                    pf = psB.tile([32, 512], F32, name="pf", tag="dense")
                    nc.tensor.matmul(pf[:, :n], w("Wf1"), p_t[:, :n],
                                     start=True, stop=True)
                    f1 = wp.tile([32, 512], BF16, name="f1", tag="f1", bufs=2)
                    leaky(f1[:, :n], pf[:, :n], bias=wf("bf1c"))

                    for sub in range(n // 128):
                        po = psB.tile([128, 2], F32, name="po", tag="dense")
                        nc.tensor.matmul(po[:], f1[:, sub * 128:(sub + 1) * 128],
                                         w("Wf2"), start=True, stop=False)
                        nc.tensor.matmul(po[:], w("ones", 0, 128), w("bf2r"),
                                         start=False, stop=True)
                        ot = wp.tile([128, 2], F32, name="ot", tag="ot")
                        nc.scalar.activation(ot[:], po[:], AF.Copy)
                        s0 = c0 + sub * 128
                        nc.sync.dma_start(out_d.ap()[s0:s0 + 128, :], ot[:])

            agg_phase(64, 128, t3_lo[:], t3_hi[:], idx_t,
                      drain3, head, "3")

    nc.compile()
    return nc


def make_core_inputs(cfg, sched, percore, inp):
    """Build per-core in_maps from the problem inputs dict."""
    C = cfg["n_cores"]
    SP = cfg["slice_pad"]
    SL = cfg["slice"]
    NN = cfg["n_nodes"]
    BFNP = mybir.dt.np(BF16)
    x = np.asarray(inp["x"], np.float32)
    iota = np.tile(np.arange(W, dtype=np.float32), (128, 1))

    def bf(a):
        return np.ascontiguousarray(np.asarray(a, np.float32)).astype(BFNP)

    xpid = np.zeros((cfg["pid_n"], 128), np.float32)
    xpid[pid_of(np.arange(NN), cfg)] = x

    # packed bf16 constants
    wvals_bf = dict(
        iota=iota,
        W1l=np.asarray(inp["W1l"], np.float32),
        W1r=np.asarray(inp["W1r"], np.float32),
        Wp=np.asarray(inp["Wp"], np.float32),
        Wf1=np.asarray(inp["Wf1"], np.float32),
        Wf2=np.asarray(inp["Wf2"], np.float32),
        ones=np.ones((1, 512), np.float32),
        bf2r=np.asarray(inp["bf2"], np.float32).reshape(1, -1),
    )
    for k in range(4):
        wvals_bf[f"W2l{k}"] = np.asarray(inp["W2l"], np.float32)[k * 128:(k + 1) * 128]
        wvals_bf[f"W2r{k}"] = np.asarray(inp["W2r"], np.float32)[k * 128:(k + 1) * 128]
    for k in range(2):
        wvals_bf[f"W3l{k}"] = np.asarray(inp["W3l"], np.float32)[k * 128:(k + 1) * 128]
        wvals_bf[f"W3r{k}"] = np.asarray(inp["W3r"], np.float32)[k * 128:(k + 1) * 128]
    wb = np.zeros((128, BF_COLS), np.float32)
    for name, (o, p, cc) in BF_OFFS.items():
        wb[0:p, o:o + cc] = wvals_bf[name]

    wvals_f32 = dict(
        b1c=np.asarray(inp["b1"], np.float32).reshape(4, 128).T,
        b2c=np.asarray(inp["b2"], np.float32).reshape(2, 128).T,
        b3c=np.asarray(inp["b3"], np.float32).reshape(64, 1),
        bpc=np.asarray(inp["bp"], np.float32).reshape(32, 1),
        bf1c=np.asarray(inp["bf1"], np.float32).reshape(32, 1),
    )

    shared = dict(wb=wb.astype(BFNP), xpid=xpid.astype(BFNP))
    in_maps = []
    for c in range(C):
        xs = np.zeros((128, SP), np.float32)
        xs[:, :SL] = x[c * SL:(c + 1) * SL].T
        ntt = len(percore[c]["seg"]) // W
        wfp = np.zeros((128, 2 * ntt + F32_COLS), np.float32)
        wfp[:, :ntt] = pack_tilewise(percore[c]["seg"], ntt)
        wfp[:, ntt:2 * ntt] = pack_tilewise(percore[c]["wgt"], ntt)
        for name, (o, p, cc) in F32_OFFS.items():
            wfp[0:p, 2 * ntt + o:2 * ntt + o + cc] = wvals_f32[name]
        m = dict(shared)
        m.update(
            xT=xs.astype(BFNP),
            idx=pack_gather_idx(percore[c]["idx"]),
            wf=wfp,
        )
        in_maps.append(m)
    return in_maps


# ----------------------------------------------------------------------
# public entry point
# ----------------------------------------------------------------------
_CACHE = {}


def _get_compiled(edge_index):
    key = hash(edge_index.tobytes())
    if key not in _CACHE:
        cfg = CFG_FULL
        sched, percore = build_structure(edge_index, cfg)
        nc = build_kernel(cfg, sched)
        _CACHE[key] = (cfg, sched, percore, nc)
    return _CACHE[key]


def _run(inputs, trace=False):
    inputs = {k: np.asarray(v) for k, v in inputs.items()}
    edge_index = np.asarray(inputs["edge_index"], np.int32)
    cfg, sched, percore, nc = _get_compiled(edge_index)
    in_maps = make_core_inputs(cfg, sched, percore, inputs)
    res = run_bass_kernel_spmd(nc, in_maps, core_ids=list(range(cfg["n_cores"])),
                               trace=trace)
    out = np.concatenate([res.results[c]["out"][:cfg["slice"]]
                          for c in range(cfg["n_cores"])]).astype(np.float32)
    return out, res


def kernel(**inputs):
    out, _ = _run(inputs)
    return out


# revision 49
# speedup vs baseline: 1.0398x; 1.0398x over previous
"""Trainium2 Bass kernel for nn_BindingSiteGraphSAGE (3-layer GraphSAGE + MLP head).

Self-contained: takes the full inputs, shards destination nodes across the
8 NeuronCores, runs a single SPMD Bass program (edge aggregation via
indirect-DMA gathers + weighted-onehot PSUM matmuls, with per-superblock
interleaved dense phases and per-superblock AllGather collectives between
layers), and returns the full [50000, 2] float32 output.

Optimizations over the first working version:
- all dense matmuls run in bf16 (fp32 costs 4 cycles/row on the PE).
- onehot builds use a bf16 iota so the DVE runs in its 2x perf mode.
- gather chunks are one (block, half) group each with num_idxs trimmed to
  the real edge count, so tile-padding slots are never fetched from HBM
  (the onehot masks the stale SBUF contents).
- leaky ReLU is a single Prelu activation (alpha=0.15).
- the L2/L3 self terms stay resident in SBUF (no DRAM roundtrip).
- gather tables (x copy, t2, t3) share one superblock-major row layout
  ("pid" space): each superblock's AllGather output is contiguous, so the
  AllGathers run per superblock as soon as that superblock's table rows are
  computed, overlapping the remaining compute; only the last small chunk's
  latency is exposed. One gather-index table serves all three layers.

Host preprocessing only reorders/pads the edge list and ships structural
metadata (indices, degree weights, schedules, a row-permuted copy of x) —
no feature math on host.
"""
import sys
for _p in ("/opt/trn_rl_repo",):
    if _p not in sys.path:
        sys.path.insert(0, _p)
import numpy as np
import concourse.bass as bass
import concourse.bacc as bacc
import concourse.tile as tile
import concourse.mybir as mybir
from concourse.bass_utils import run_bass_kernel_spmd

"""Host-side graph structure preprocessing.

Partitions dst nodes across cores, builds a uniform (core-independent)
tile schedule for edge aggregation, and per-core gather/onehot buffers.

Layout conventions:
- dst slice per core: cfg['slice'] real nodes, padded to cfg['slice_pad'].
- dst blocks of W=128 local dsts; superblocks per cfg['sb_sizes'].
- gather-table rows live in "pid" space, superblock-major:
  pid = 8192*sb + core*sbc(sb) + (local_row - 1024*sb); superblocks 0-3
  occupy pids [0, 32768) ("lo"), 4-6 occupy [32768, 50176) ("hi"), so
  int16 gather indices are pid or pid - 32768.
- edges split lo/hi by the src's superblock (local_row < 4096).
- per (sb, half, block): fixed tile count = max over cores (>=1 for lo).
- gather idx i -> partition i%16, free i//16 (replicated x8 over partitions).
- seg/wgt per edge-slot: partition i%128, free tile index (seg 128 = pad).
"""

W = 128


def make_cfg(n_cores, n_nodes, slice_, slice_pad, sb_sizes):
    assert slice_pad % W == 0 and sum(sb_sizes) == slice_pad // W
    assert sb_sizes == [8, 8, 8, 8, 8, 8, 1], "pid map hardcodes this layout"
    return dict(n_cores=n_cores, n_nodes=n_nodes, slice=slice_,
                slice_pad=slice_pad, sb_sizes=sb_sizes,
                half_r=4096, hi_base=32768, n_lo_sb=4,
                pid_n=n_cores * slice_pad)


CFG_FULL = make_cfg(8, 50000, 6250, 6272, [8, 8, 8, 8, 8, 8, 1])

# gather chunks are (block, half) groups, merged up to this many tiles while
# still ending on a group boundary (the trailing group's padding is skipped
# via num_idxs). hi-half groups are smaller (35% of edges) which makes their
# gathers Pool-desc-gen-bound, so they merge in pairs.
MAX_CHUNK_TILES = 24
GROUPS_PER_CHUNK = {0: 1, 1: 2}


def pid_of(src, cfg):
    """node id -> row in the superblock-major gather tables (vectorized)."""
    src = np.asarray(src, dtype=np.int64)
    SL, C = cfg["slice"], cfg["n_cores"]
    r = src % SL
    c = src // SL
    s = r // 1024                      # 6144..6249 -> 6 (last, short sb)
    sbc = np.where(s < 6, 1024, 128)
    return 8192 * s + c * sbc + (r - 1024 * s)


def build_structure(edge_index, cfg):
    src = np.asarray(edge_index[0], dtype=np.int64)
    dst = np.asarray(edge_index[1], dtype=np.int64)
    C, SL, SP = cfg["n_cores"], cfg["slice"], cfg["slice_pad"]
    HALF_R, HI_BASE = cfg["half_r"], cfg["hi_base"]

    percore_edges = []
    for c in range(C):
        m = (dst >= c * SL) & (dst < (c + 1) * SL)
        es, ed = src[m], dst[m] - c * SL
        cnt = np.bincount(ed, minlength=SP).astype(np.float32)
        wall = 1.0 / np.maximum(cnt, 1.0)
        percore_edges.append((es, ed, wall))

    sched = []
    percore = [dict(idx=[], seg=[], wgt=[]) for _ in range(C)]
    b0 = 0
    for sb_sz in cfg["sb_sizes"]:
        blocks = list(range(b0, b0 + sb_sz))
        ntiles = {0: [], 1: []}
        maxe = {0: [], 1: []}
        elists = {0: [], 1: []}
        for h in (0, 1):
            for b in blocks:
                per_core_list = []
                mx = 1 if h == 0 else 0
                me = 0
                for c in range(C):
                    es, ed, wall = percore_edges[c]
                    hlo = (es % SL) < HALF_R
                    m = (ed >= b * W) & (ed < (b + 1) * W) & \
                        (hlo if h == 0 else ~hlo)
                    per_core_list.append((es[m], ed[m]))
                    mx = max(mx, (len(es[m]) + W - 1) // W)
                    me = max(me, len(es[m]))
                ntiles[h].append(mx)
                maxe[h].append(me)
                elists[h].append(per_core_list)

        tiles = []   # (half, b_local)
        for h in (0, 1):
            for bl in range(sb_sz):
                tiles += [(h, bl)] * ntiles[h][bl]

        for c in range(C):
            for h in (0, 1):
                for bl in range(sb_sz):
                    es, ed = elists[h][bl][c]
                    wall = percore_edges[c][2]
                    nslot = ntiles[h][bl] * W
                    ix = np.zeros(nslot, np.int16)
                    sg = np.full(nslot, W, np.float32)
                    wg = np.zeros(nslot, np.float32)
                    ne = len(es)
                    assert ne <= nslot
                    pv = pid_of(es, cfg)
                    if h == 1:
                        pv = pv - HI_BASE
                    assert ne == 0 or (pv.min() >= 0 and pv.max() < 32768)
                    ix[:ne] = pv.astype(np.int16)
                    sg[:ne] = (ed - blocks[bl] * W).astype(np.float32)
                    wg[:ne] = wall[ed]
                    percore[c]["idx"].append(ix)
                    percore[c]["seg"].append(sg)
                    percore[c]["wgt"].append(wg)

        # chunks: merge up to GROUPS_PER_CHUNK (block, half) groups while
        # <= MAX_CHUNK_TILES tiles; num_idxs trimmed to skip the trailing
        # group's padding slots.
        chunks = []
        t = 0
        for h in (0, 1):
            run = []  # (nt, maxE) for groups pending merge
            for bl in range(sb_sz):
                nt_g, me_g = ntiles[h][bl], maxe[h][bl]
                if run and (len(run) >= GROUPS_PER_CHUNK[h] or
                            sum(r[0] for r in run) + nt_g > MAX_CHUNK_TILES):
                    tot = sum(r[0] for r in run)
                    lead = tot - run[-1][0]
                    nidx = lead * W + max(run[-1][1], 1)
                    nidx = min(tot * W, 16 * ((nidx + 15) // 16))
                    nidx = max(nidx, (tot - 1) * W + 16)
                    chunks.append((h, t, t + tot, nidx))
                    t += tot
                    run = []
                run.append((nt_g, me_g))
            if run:
                tot = sum(r[0] for r in run)
                lead = tot - run[-1][0]
                nidx = lead * W + max(run[-1][1], 1)
                nidx = min(tot * W, 16 * ((nidx + 15) // 16))
                nidx = max(nidx, (tot - 1) * W + 16)
                chunks.append((h, t, t + tot, nidx))
                t += tot
        assert t == len(tiles)
        sched.append(dict(ntiles=ntiles, tiles=tiles, chunks=chunks,
                          blocks=blocks, sb_sz=sb_sz))
        b0 += sb_sz

    for c in range(C):
        for k in ("idx", "seg", "wgt"):
            percore[c][k] = np.concatenate(percore[c][k])
    return sched, percore


def pack_gather_idx(flat_idx):
    """[n] int16 -> [128, n/16] buffer (16-partition wrap, replicated x8)."""
    n = len(flat_idx)
    assert n % 16 == 0
    b = flat_idx.reshape(n // 16, 16).T
    return np.tile(b, (8, 1)).copy()


def pack_tilewise(flat, ntiles):
    """[n=ntiles*128] -> [128, ntiles] (edge i -> part i%128, free i//128)."""
    return flat.reshape(ntiles, W).T.copy()


F32 = mybir.dt.float32
BF16 = mybir.dt.bfloat16
AF = mybir.ActivationFunctionType
ALU = mybir.AluOpType
W = 128
SLOPE = 0.15

# all small constants ship as two packed tensors (one bf16, one f32 that also
# carries seg/wgt) so startup is 2 big DMAs instead of ~24 serialized HWDGE ops
WPACK_BF = ([("iota", 128, 128), ("W1l", 128, 512), ("W1r", 128, 512)]
            + [(f"W2l{k}", 128, 256) for k in range(4)]
            + [(f"W2r{k}", 128, 256) for k in range(4)]
            + [(f"W3l{k}", 128, 64) for k in range(2)]
            + [(f"W3r{k}", 128, 64) for k in range(2)]
            + [("Wp", 64, 32), ("Wf1", 32, 32), ("Wf2", 32, 2),
               ("ones", 1, 512), ("bf2r", 1, 2)])
WPACK_F32 = [("b1c", 128, 4), ("b2c", 128, 2), ("b3c", 64, 1),
             ("bpc", 32, 1), ("bf1c", 32, 1)]


def pack_offsets(spec):
    offs, o = {}, 0
    for name, part, cols in spec:
        offs[name] = (o, part, cols)
        o += cols
    return offs, o


BF_OFFS, BF_COLS = pack_offsets(WPACK_BF)
F32_OFFS, F32_COLS = pack_offsets(WPACK_F32)


def ceil_div(a, b):
    return (a + b - 1) // b


def build_kernel(cfg, sched, timing_mode=False):
    """cfg: dict(n_cores, n_nodes, slice, slice_pad, sb_sizes, ...)
    sched: from build_structure (list of superblock dicts)."""
    C = cfg["n_cores"]
    SP = cfg["slice_pad"]
    HI_BASE = cfg["hi_base"]
    N_LO_SB = cfg["n_lo_sb"]
    PIDN = cfg["pid_n"]
    NTT = sum(len(s["tiles"]) for s in sched)      # total tiles
    NID = NTT * W                                   # total edge slots
    MAXSBC = max(s["sb_sz"] for s in sched) * W
    MAXNT = max(t1 - t0 for s in sched for (_, t0, t1, _) in s["chunks"])
    sb_rows = [C * s["sb_sz"] * W for s in sched]   # pid rows per superblock
    sb_row0 = np.cumsum([0] + sb_rows).tolist()
    HI_ROWS = PIDN - HI_BASE

    def sb_chunks(sbc):
        return [(i * 512, min(512, sbc - i * 512)) for i in range(ceil_div(sbc, 512))]

    nc = bacc.Bacc("TRN2", target_bir_lowering=False, debug=False,
                   enable_asserts=True, num_devices=(1 if timing_mode else C))

    # ---------------- DRAM I/O ----------------
    xpid_d = nc.dram_tensor("xpid", [PIDN, 128], BF16, kind="ExternalInput")
    xT_d = nc.dram_tensor("xT", [128, SP], BF16, kind="ExternalInput")
    idx_d = nc.dram_tensor("idx", [128, NID // 16], mybir.dt.int16, kind="ExternalInput")
    wb_d = nc.dram_tensor("wb", [128, BF_COLS], BF16, kind="ExternalInput")
    wf_d = nc.dram_tensor("wf", [128, 2 * NTT + F32_COLS], F32, kind="ExternalInput")
    out_d = nc.dram_tensor("out", [SP, 2], F32, kind="ExternalOutput")

    with tile.TileContext(nc) as tc:
        with (
            tc.tile_pool(name="const", bufs=1) as cp,
            tc.tile_pool(name="big", bufs=1) as bp,       # long-lived buffers
            tc.tile_pool(name="gath", bufs=6) as gp,      # gather chunks
            tc.tile_pool(name="work", bufs=16) as wp,      # onehot / drains / stages
            tc.tile_pool(name="psA", bufs=1, space="PSUM") as psA,   # agg psums
            tc.tile_pool(name="psB", bufs=2, space="PSUM") as psB,   # dense psums
            tc.tile_pool(name="dram", bufs=1, space="DRAM") as dp,
        ):
            # ---------------- constants ----------------
            wf_t = cp.tile([128, 2 * NTT + F32_COLS], F32, name="wf_t")
            nc.sync.dma_start(wf_t[:], wf_d.ap())
            wb_t = cp.tile([128, BF_COLS], BF16, name="wb_t")
            nc.sync.dma_start(wb_t[:], wb_d.ap())

            def w(name, c0=None, c1=None):
                o, p, c = BF_OFFS[name]
                if c0 is None:
                    c0, c1 = 0, c
                return wb_t[0:p, o + c0:o + c1]

            def wf(name, c0=None, c1=None):
                o, p, c = F32_OFFS[name]
                if c0 is None:
                    c0, c1 = 0, c
                return wf_t[0:p, 2 * NTT + o + c0:2 * NTT + o + c1]

            # persistent self terms (SBUF-resident)
            selfb2 = [cp.tile([128, SP], BF16, name=f"selfb2_{m}") for m in range(2)]
            selfb3 = cp.tile([64, SP], BF16)

            # DRAM scratch: allgathered tables, split lo/hi so gathers from
            # the lo pids only depend on the lo superblocks' collectives
            t2_lo = dp.tile([HI_BASE, 256], BF16)
            t2_hi = dp.tile([HI_ROWS, 256], BF16)
            t3_lo = dp.tile([HI_BASE, 128], BF16)
            t3_hi = dp.tile([HI_ROWS, 128], BF16)
            t2_slice = dp.tile([SP, 256], BF16)
            t3_slice = dp.tile([SP, 128], BF16)

            def emit_ag(si, slice_t, lo_t, hi_t):
                if timing_mode:
                    return
                col0 = sched[si]["blocks"][0] * W
                sbc = sched[si]["sb_sz"] * W
                r0 = sb_row0[si]
                dst_t, doff = (lo_t, 0) if si < N_LO_SB else (hi_t, HI_BASE)
                nc.gpsimd.collective_compute(
                    "AllGather", ALU.bypass, replica_groups=[list(range(C))],
                    ins=[slice_t[col0:col0 + sbc, :].opt()],
                    outs=[dst_t[r0 - doff:r0 - doff + C * sbc, :].opt()])

            # prime the gather-pool buffers so slots skipped by trimmed
            # num_idxs never expose uninitialized SBUF (NaN x 0 = NaN in PE)
            for pi in range(6):
                gz = gp.tile([128, MAXNT * 256], BF16, name=f"gz{pi}", tag="g")
                nc.vector.memset(gz[:], 0.0)

            def leaky(dst_ap, src_ap, bias=0.0):
                # Prelu honors alpha on HW (Lrelu ignores it: fixed slope)
                nc.scalar.activation(dst_ap, src_ap, AF.Prelu, bias=bias,
                                     alpha=SLOPE)

            # =========== generic aggregation phase ===========
            def agg_phase(d, row_elems, lo_ap, hi_ap, idx_t, drain_fn,
                          after_sb, tag):
                """d: used dims; row_elems: table row width.
                drain_fn(si, mp, ps_ap, col0, ncols): consume psum [dmp, ncols]
                holding agg_T rows [mp*128, ...) cols [col0, col0+ncols).
                after_sb(si, col0, sbc): emit dependent work for the sb."""
                n_mp = ceil_div(d, 128)
                dmp = min(d, 128)
                toff = 0
                for si, s in enumerate(sched):
                    sbc = s["sb_sz"] * W
                    tiles = s["tiles"]
                    bank_of = lambda bl: (bl * W) // 512
                    first, last = {}, {}
                    for i, (h, bl) in enumerate(tiles):
                        bk = bank_of(bl)
                        first.setdefault(bk, i)
                        last[bk] = i
                    ps = [psA.tile([dmp, sbc], F32, name=f"ps{tag}{mp}",
                                   tag=f"aggps{mp}", bufs=(2 if mp == 0 else 1))
                          for mp in range(n_mp)]
                    for (h, t0, t1, nidx) in s["chunks"]:
                        nt = t1 - t0
                        g = gp.tile([128, nt, row_elems], BF16, name=f"g{tag}", tag="g")
                        base = lo_ap if h == 0 else hi_ap
                        nc.gpsimd.dma_gather(
                            g[:, :nt, :], base,
                            idx_t[:, (toff + t0) * 8:(toff + t1) * 8],
                            nidx, nidx, elem_size=row_elems, single_packet=False)
                        for t in range(t0, t1):
                            h_, bl = tiles[t]
                            bk = bank_of(bl)
                            oh = wp.tile([128, W], BF16, name=f"oh{tag}", tag="oh", bufs=16)
                            gt = toff + t
                            nc.vector.tensor_scalar(
                                oh[:], w("iota"), wf_t[:, gt:gt + 1],
                                wf_t[:, NTT + gt:NTT + gt + 1],
                                ALU.is_equal, ALU.mult)
                            for mp in range(n_mp):
                                nc.tensor.matmul(
                                    ps[mp][:, bl * W:(bl + 1) * W],
                                    g[:, t - t0, mp * dmp:(mp + 1) * dmp],
                                    oh[:],
                                    start=(first[bk] == t), stop=(last[bk] == t))
                    col0 = s["blocks"][0] * W
                    # drain the single-buffered mps first: the next
                    # superblock's matmuls wait on their psum buffers
                    for mp in reversed(range(n_mp)):
                        cc = 0
                        while cc < sbc:
                            n = min(512, sbc - cc)
                            drain_fn(si, mp, ps[mp][:, cc:cc + n], col0 + cc, n)
                            cc += n
                    if after_sb is not None:
                        after_sb(si, col0, sbc)
                    toff += len(tiles)

            # =========== L1: agg + interleaved dense ===========
            # only the first superblock's indices load upfront; the rest (and
            # most of xT) load behind the first superblock's gathers so the
            # DMA engines start useful work immediately
            idx_t = bp.tile([128, NID // 16], mybir.dt.int16, name="idx_t", tag="big1")
            NT0 = len(sched[0]["tiles"])
            SBC0 = sched[0]["sb_sz"] * W
            nc.sync.dma_start(idx_t[:, :NT0 * 8], idx_d.ap()[:, :NT0 * 8])
            xT_t = bp.tile([128, SP], BF16, name="xT_t", tag="big3")
            nc.sync.dma_start(xT_t[:, :SBC0], xT_d.ap()[:, :SBC0])

            agg1_sb = {}

            def drain1(si, mp, ps_ap, c0, n):
                if si not in agg1_sb:
                    agg1_sb[si] = (bp.tile([128, MAXSBC], BF16, name=f"agg1_{si}",
                                           tag="agg1", bufs=3),
                                   sched[si]["blocks"][0] * W)
                t_, col0 = agg1_sb[si]
                nc.scalar.activation(t_[:, c0 - col0:c0 - col0 + n], ps_ap, AF.Copy)

            def l1_dense(si, col0, sbc):
                if si == 0:
                    nc.sync.dma_start(idx_t[:, NT0 * 8:], idx_d.ap()[:, NT0 * 8:])
                    nc.sync.dma_start(xT_t[:, SBC0:], xT_d.ap()[:, SBC0:])
                a1, _ = agg1_sb.pop(si)
                for (off, n) in sb_chunks(sbc):
                    c0 = col0 + off
                    h1 = [wp.tile([128, 512], BF16, name=f"h1_{m}", tag=f"h1_{m}",
                                  bufs=2) for m in range(4)]
                    for m in range(4):
                        ph = psB.tile([128, 512], F32, name="ph1", tag="dense")
                        nc.tensor.matmul(ph[:, :n], w("W1l", m * 128, (m + 1) * 128),
                                         a1[:, off:off + n], start=True, stop=False)
                        nc.tensor.matmul(ph[:, :n], w("W1r", m * 128, (m + 1) * 128),
                                         xT_t[:, c0:c0 + n], start=False, stop=True)
                        leaky(h1[m][:, :n], ph[:, :n], bias=wf("b1c", m, m + 1))
                    for sub in range(n // 128):
                        pt = psB.tile([128, 256], F32, name="pt2", tag="dense")
                        for k in range(4):
                            nc.tensor.matmul(pt[:], h1[k][:, sub * 128:(sub + 1) * 128],
                                             w(f"W2l{k}"),
                                             start=(k == 0), stop=(k == 3))
                        st = wp.tile([128, 256], BF16, name="st2", tag="st2")
                        nc.scalar.activation(st[:], pt[:], AF.Copy)
                        r0 = c0 + sub * 128
                        nc.sync.dma_start(t2_slice[r0:r0 + 128, :], st[:])
                    for m in range(2):
                        pss = psB.tile([128, 512], F32, name="ps2", tag="dense")
                        for k in range(4):
                            nc.tensor.matmul(pss[:, :n],
                                             w(f"W2r{k}", m * 128, (m + 1) * 128),
                                             h1[k][:, :n], start=(k == 0), stop=(k == 3))
                        nc.scalar.activation(selfb2[m][:, c0:c0 + n], pss[:, :n],
                                             AF.Identity, bias=wf("b2c", m, m + 1))
                emit_ag(si, t2_slice, t2_lo, t2_hi)

            agg_phase(128, 128, xpid_d.ap(), xpid_d.ap()[HI_BASE:, :],
                      idx_t, drain1, l1_dense, "1")

            # =========== L2 ===========
            h2_sb = {}

            def drain2(si, mp, ps_ap, c0, n):
                key = (si, mp)
                if key not in h2_sb:
                    h2_sb[key] = (bp.tile([128, MAXSBC], BF16, name=f"h2_{si}_{mp}",
                                          tag=f"h2m{mp}", bufs=2),
                                  sched[si]["blocks"][0] * W)
                t_, col0 = h2_sb[key]
                pre = wp.tile([128, 512], BF16, name="pre2", tag="pre")
                nc.vector.tensor_tensor(pre[:, :n], ps_ap,
                                        selfb2[mp][:, c0:c0 + n], ALU.add)
                leaky(t_[:, c0 - col0:c0 - col0 + n], pre[:, :n])

            def l2_dense(si, col0, sbc):
                h2m = [h2_sb.pop((si, mp))[0] for mp in range(2)]
                for sub in range(sbc // 128):
                    pt = psB.tile([128, 64], F32, name="pt3", tag="dense")
                    s0 = col0 + sub * 128
                    for k in range(2):
                        nc.tensor.matmul(pt[:], h2m[k][:, sub * 128:(sub + 1) * 128],
                                         w(f"W3l{k}"),
                                         start=(k == 0), stop=(k == 1))
                    st = wp.tile([128, 64], BF16, name="st3", tag="st3")
                    nc.scalar.activation(st[:], pt[:], AF.Copy)
                    nc.sync.dma_start(t3_slice[s0:s0 + 128, :64], st[:])
                for (off, n) in sb_chunks(sbc):
                    c0 = col0 + off
                    pss = psB.tile([64, 512], F32, name="ps3", tag="dense")
                    for k in range(2):
                        nc.tensor.matmul(pss[:, :n], w(f"W3r{k}"),
                                         h2m[k][:, off:off + n],
                                         start=(k == 0), stop=(k == 1))
                    nc.scalar.activation(selfb3[:, c0:c0 + n], pss[:, :n],
                                         AF.Identity, bias=wf("b3c"))
                emit_ag(si, t3_slice, t3_lo, t3_hi)

            agg_phase(256, 256, t2_lo[:], t2_hi[:], idx_t,
                      drain2, l2_dense, "2")

            # =========== L3 + head ===========
            h3_sb = {}

            def drain3(si, mp, ps_ap, c0, n):
                if si not in h3_sb:
                    h3_sb[si] = (bp.tile([64, MAXSBC], BF16, name=f"h3_{si}",
                                         tag="h3", bufs=2),
                                 sched[si]["blocks"][0] * W)
                t_, col0 = h3_sb[si]
                pre = wp.tile([64, 512], BF16, name="pre3", tag="pre")
                nc.vector.tensor_tensor(pre[:, :n], ps_ap,
                                        selfb3[:, c0:c0 + n], ALU.add)
                leaky(t_[:, c0 - col0:c0 - col0 + n], pre[:, :n])

            def head(si, col0, sbc):
                h3t, _ = h3_sb.pop(si)
                for (off, n) in sb_chunks(sbc):
                    c0 = col0 + off
                    pp = psB.tile([32, 512], F32, name="pp", tag="dense")
                    nc.tensor.matmul(pp[:, :n], w("Wp"), h3t[:, off:off + n],
                                     start=True, stop=True)
                    p_t = wp.tile([32, 512], BF16, name="p", tag="p", bufs=2)
                    nc.scalar.activation(p_t[:, :n], pp[:, :n], AF.Identity,
                                         bias=wf("bpc"))

                    pf = psB.tile([32, 512], F32, name="pf", tag="dense")
                    nc.tensor.matmul(pf[:, :n], w("Wf1"), p_t[:, :n],
                                     start=True, stop=True)
                    f1 = wp.tile([32, 512], BF16, name="f1", tag="f1", bufs=2)
                    leaky(f1[:, :n], pf[:, :n], bias=wf("bf1c"))

                    for sub in range(n // 128):
                        po = psB.tile([128, 2], F32, name="po", tag="dense")
                        nc.tensor.matmul(po[:], f1[:, sub * 128:(sub + 1) * 128],
                                         w("Wf2"), start=True, stop=False)
                        nc.tensor.matmul(po[:], w("ones", 0, 128), w("bf2r"),
                                         start=False, stop=True)
                        ot = wp.tile([128, 2], F32, name="ot", tag="ot")
                        nc.scalar.activation(ot[:], po[:], AF.Copy)
                        s0 = c0 + sub * 128
                        nc.sync.dma_start(out_d.ap()[s0:s0 + 128, :], ot[:])

            agg_phase(64, 128, t3_lo[:], t3_hi[:], idx_t,
                      drain3, head, "3")

    nc.compile()
    return nc


def make_core_inputs(cfg, sched, percore, inp):
    """Build per-core in_maps from the problem inputs dict."""
    C = cfg["n_cores"]
    SP = cfg["slice_pad"]
    SL = cfg["slice"]
    NN = cfg["n_nodes"]
    BFNP = mybir.dt.np(BF16)
    x = np.asarray(inp["x"], np.float32)
    iota = np.tile(np.arange(W, dtype=np.float32), (128, 1))

    def bf(a):
        return np.ascontiguousarray(np.asarray(a, np.float32)).astype(BFNP)

    xpid = np.zeros((cfg["pid_n"], 128), np.float32)
    xpid[pid_of(np.arange(NN), cfg)] = x

    # packed bf16 constants
    wvals_bf = dict(
        iota=iota,
        W1l=np.asarray(inp["W1l"], np.float32),
        W1r=np.asarray(inp["W1r"], np.float32),
        Wp=np.asarray(inp["Wp"], np.float32),
        Wf1=np.asarray(inp["Wf1"], np.float32),
        Wf2=np.asarray(inp["Wf2"], np.float32),
        ones=np.ones((1, 512), np.float32),
        bf2r=np.asarray(inp["bf2"], np.float32).reshape(1, -1),
    )
    for k in range(4):
        wvals_bf[f"W2l{k}"] = np.asarray(inp["W2l"], np.float32)[k * 128:(k + 1) * 128]
        wvals_bf[f"W2r{k}"] = np.asarray(inp["W2r"], np.float32)[k * 128:(k + 1) * 128]
    for k in range(2):
        wvals_bf[f"W3l{k}"] = np.asarray(inp["W3l"], np.float32)[k * 128:(k + 1) * 128]
        wvals_bf[f"W3r{k}"] = np.asarray(inp["W3r"], np.float32)[k * 128:(k + 1) * 128]
    wb = np.zeros((128, BF_COLS), np.float32)
    for name, (o, p, cc) in BF_OFFS.items():
        wb[0:p, o:o + cc] = wvals_bf[name]

    wvals_f32 = dict(
        b1c=np.asarray(inp["b1"], np.float32).reshape(4, 128).T,
        b2c=np.asarray(inp["b2"], np.float32).reshape(2, 128).T,
        b3c=np.asarray(inp["b3"], np.float32).reshape(64, 1),
        bpc=np.asarray(inp["bp"], np.float32).reshape(32, 1),
        bf1c=np.asarray(inp["bf1"], np.float32).reshape(32, 1),
    )

    shared = dict(wb=wb.astype(BFNP), xpid=xpid.astype(BFNP))
    in_maps = []
    for c in range(C):
        xs = np.zeros((128, SP), np.float32)
        xs[:, :SL] = x[c * SL:(c + 1) * SL].T
        ntt = len(percore[c]["seg"]) // W
        wfp = np.zeros((128, 2 * ntt + F32_COLS), np.float32)
        wfp[:, :ntt] = pack_tilewise(percore[c]["seg"], ntt)
        wfp[:, ntt:2 * ntt] = pack_tilewise(percore[c]["wgt"], ntt)
        for name, (o, p, cc) in F32_OFFS.items():
            wfp[0:p, 2 * ntt + o:2 * ntt + o + cc] = wvals_f32[name]
        m = dict(shared)
        m.update(
            xT=xs.astype(BFNP),
            idx=pack_gather_idx(percore[c]["idx"]),
            wf=wfp,
        )
        in_maps.append(m)
    return in_maps


# ----------------------------------------------------------------------
# public entry point
# ----------------------------------------------------------------------
_CACHE = {}


def _get_compiled(edge_index):
    key = hash(edge_index.tobytes())
    if key not in _CACHE:
        cfg = CFG_FULL
        sched, percore = build_structure(edge_index, cfg)
        nc = build_kernel(cfg, sched)
        _CACHE[key] = (cfg, sched, percore, nc)
    return _CACHE[key]


def _run(inputs, trace=False):
    inputs = {k: np.asarray(v) for k, v in inputs.items()}
    edge_index = np.asarray(inputs["edge_index"], np.int32)
    cfg, sched, percore, nc = _get_compiled(edge_index)
    in_maps = make_core_inputs(cfg, sched, percore, inputs)
    res = run_bass_kernel_spmd(nc, in_maps, core_ids=list(range(cfg["n_cores"])),
                               trace=trace)
    out = np.concatenate([res.results[c]["out"][:cfg["slice"]]
                          for c in range(cfg["n_cores"])]).astype(np.float32)
    return out, res


def kernel(**inputs):
    out, _ = _run(inputs)
    return out


# revision 54
# speedup vs baseline: 1.0492x; 1.0091x over previous
"""Trainium2 Bass kernel for nn_BindingSiteGraphSAGE (3-layer GraphSAGE + MLP head).

Self-contained: takes the full inputs, shards destination nodes across the
8 NeuronCores, runs a single SPMD Bass program (edge aggregation via
indirect-DMA gathers + weighted-onehot PSUM matmuls, with per-superblock
interleaved dense phases and per-superblock AllGather collectives between
layers), and returns the full [50000, 2] float32 output.

Optimizations over the first working version:
- all dense matmuls run in bf16 (fp32 costs 4 cycles/row on the PE).
- onehot builds use a bf16 iota so the DVE runs in its 2x perf mode.
- gather chunks are one (block, half) group each with num_idxs trimmed to
  the real edge count, so tile-padding slots are never fetched from HBM
  (the onehot masks the stale SBUF contents).
- leaky ReLU is a single Prelu activation (alpha=0.15).
- the L2/L3 self terms stay resident in SBUF (no DRAM roundtrip).
- gather tables (x copy, t2, t3) share one superblock-major row layout
  ("pid" space): each superblock's AllGather output is contiguous, so the
  AllGathers run per superblock as soon as that superblock's table rows are
  computed, overlapping the remaining compute; only the last small chunk's
  latency is exposed. One gather-index table serves all three layers.

Host preprocessing only reorders/pads the edge list and ships structural
metadata (indices, degree weights, schedules, a row-permuted copy of x) —
no feature math on host.
"""
import sys
for _p in ("/opt/trn_rl_repo",):
    if _p not in sys.path:
        sys.path.insert(0, _p)
import numpy as np
import concourse.bass as bass
import concourse.bacc as bacc
import concourse.tile as tile
import concourse.mybir as mybir
from concourse.bass_utils import run_bass_kernel_spmd

"""Host-side graph structure preprocessing.

Partitions dst nodes across cores, builds a uniform (core-independent)
tile schedule for edge aggregation, and per-core gather/onehot buffers.

Layout conventions:
- dst slice per core: cfg['slice'] real nodes, padded to cfg['slice_pad'].
- dst blocks of W=128 local dsts; superblocks per cfg['sb_sizes'].
- gather-table rows live in "pid" space, superblock-major:
  pid = 8192*sb + core*sbc(sb) + (local_row - 1024*sb); superblocks 0-3
  occupy pids [0, 32768) ("lo"), 4-6 occupy [32768, 50176) ("hi"), so
  int16 gather indices are pid or pid - 32768.
- edges split lo/hi by the src's superblock (local_row < 4096).
- per (sb, half, block): fixed tile count = max over cores (>=1 for lo).
- gather idx i -> partition i%16, free i//16 (replicated x8 over partitions).
- seg/wgt per edge-slot: partition i%128, free tile index (seg 128 = pad).
"""

W = 128


def make_cfg(n_cores, n_nodes, slice_, slice_pad, sb_sizes):
    assert slice_pad % W == 0 and sum(sb_sizes) == slice_pad // W
    assert sb_sizes == [4] * 12 + [1], "pid map hardcodes this layout"
    return dict(n_cores=n_cores, n_nodes=n_nodes, slice=slice_,
                slice_pad=slice_pad, sb_sizes=sb_sizes,
                half_r=4096, hi_base=32768, n_lo_sb=8,
                pid_n=n_cores * slice_pad)


CFG_FULL = make_cfg(8, 50000, 6250, 6272, [4] * 12 + [1])

# gather chunks are (block, half) groups, merged up to this many tiles while
# still ending on a group boundary (the trailing group's padding is skipped
# via num_idxs). hi-half groups are smaller (35% of edges) which makes their
# gathers Pool-desc-gen-bound, so they merge in pairs.
MAX_CHUNK_TILES = 24
GROUPS_PER_CHUNK = {0: 1, 1: 2}


def pid_of(src, cfg):
    """node id -> row in the superblock-major gather tables (vectorized)."""
    src = np.asarray(src, dtype=np.int64)
    SL, C = cfg["slice"], cfg["n_cores"]
    r = src % SL
    c = src // SL
    s = r // 512                       # 6144..6249 -> 12 (last, short sb)
    sbc = np.where(s < 12, 512, 128)
    return 4096 * s + c * sbc + (r - 512 * s)


def build_structure(edge_index, cfg):
    src = np.asarray(edge_index[0], dtype=np.int64)
    dst = np.asarray(edge_index[1], dtype=np.int64)
    C, SL, SP = cfg["n_cores"], cfg["slice"], cfg["slice_pad"]
    HALF_R, HI_BASE = cfg["half_r"], cfg["hi_base"]

    percore_edges = []
    for c in range(C):
        m = (dst >= c * SL) & (dst < (c + 1) * SL)
        es, ed = src[m], dst[m] - c * SL
        cnt = np.bincount(ed, minlength=SP).astype(np.float32)
        wall = 1.0 / np.maximum(cnt, 1.0)
        percore_edges.append((es, ed, wall))

    sched = []
    percore = [dict(idx=[], seg=[], wgt=[]) for _ in range(C)]
    b0 = 0
    for sb_sz in cfg["sb_sizes"]:
        blocks = list(range(b0, b0 + sb_sz))
        ntiles = {0: [], 1: []}
        maxe = {0: [], 1: []}
        elists = {0: [], 1: []}
        for h in (0, 1):
            for b in blocks:
                per_core_list = []
                mx = 1 if h == 0 else 0
                me = 0
                for c in range(C):
                    es, ed, wall = percore_edges[c]
                    hlo = (es % SL) < HALF_R
                    m = (ed >= b * W) & (ed < (b + 1) * W) & \
                        (hlo if h == 0 else ~hlo)
                    per_core_list.append((es[m], ed[m]))
                    mx = max(mx, (len(es[m]) + W - 1) // W)
                    me = max(me, len(es[m]))
                ntiles[h].append(mx)
                maxe[h].append(me)
                elists[h].append(per_core_list)

        tiles = []   # (half, b_local)
        for h in (0, 1):
            for bl in range(sb_sz):
                tiles += [(h, bl)] * ntiles[h][bl]

        for c in range(C):
            for h in (0, 1):
                for bl in range(sb_sz):
                    es, ed = elists[h][bl][c]
                    wall = percore_edges[c][2]
                    nslot = ntiles[h][bl] * W
                    ix = np.zeros(nslot, np.int16)
                    sg = np.full(nslot, W, np.float32)
                    wg = np.zeros(nslot, np.float32)
                    ne = len(es)
                    assert ne <= nslot
                    pv = pid_of(es, cfg)
                    if h == 1:
                        pv = pv - HI_BASE
                    assert ne == 0 or (pv.min() >= 0 and pv.max() < 32768)
                    ix[:ne] = pv.astype(np.int16)
                    sg[:ne] = (ed - blocks[bl] * W).astype(np.float32)
                    wg[:ne] = wall[ed]
                    percore[c]["idx"].append(ix)
                    percore[c]["seg"].append(sg)
                    percore[c]["wgt"].append(wg)

        # chunks: merge up to GROUPS_PER_CHUNK (block, half) groups while
        # <= MAX_CHUNK_TILES tiles; num_idxs trimmed to skip the trailing
        # group's padding slots.
        chunks = []
        t = 0
        for h in (0, 1):
            run = []  # (nt, maxE) for groups pending merge
            for bl in range(sb_sz):
                nt_g, me_g = ntiles[h][bl], maxe[h][bl]
                if run and (len(run) >= GROUPS_PER_CHUNK[h] or
                            sum(r[0] for r in run) + nt_g > MAX_CHUNK_TILES):
                    tot = sum(r[0] for r in run)
                    lead = tot - run[-1][0]
                    nidx = lead * W + max(run[-1][1], 1)
                    nidx = min(tot * W, 16 * ((nidx + 15) // 16))
                    nidx = max(nidx, (tot - 1) * W + 16)
                    chunks.append((h, t, t + tot, nidx))
                    t += tot
                    run = []
                run.append((nt_g, me_g))
            if run:
                tot = sum(r[0] for r in run)
                lead = tot - run[-1][0]
                nidx = lead * W + max(run[-1][1], 1)
                nidx = min(tot * W, 16 * ((nidx + 15) // 16))
                nidx = max(nidx, (tot - 1) * W + 16)
                chunks.append((h, t, t + tot, nidx))
                t += tot
        assert t == len(tiles)
        sched.append(dict(ntiles=ntiles, tiles=tiles, chunks=chunks,
                          blocks=blocks, sb_sz=sb_sz))
        b0 += sb_sz

    for c in range(C):
        for k in ("idx", "seg", "wgt"):
            percore[c][k] = np.concatenate(percore[c][k])
    return sched, percore


def pack_gather_idx(flat_idx):
    """[n] int16 -> [128, n/16] buffer (16-partition wrap, replicated x8)."""
    n = len(flat_idx)
    assert n % 16 == 0
    b = flat_idx.reshape(n // 16, 16).T
    return np.tile(b, (8, 1)).copy()


def pack_tilewise(flat, ntiles):
    """[n=ntiles*128] -> [128, ntiles] (edge i -> part i%128, free i//128)."""
    return flat.reshape(ntiles, W).T.copy()


F32 = mybir.dt.float32
BF16 = mybir.dt.bfloat16
AF = mybir.ActivationFunctionType
ALU = mybir.AluOpType
W = 128
SLOPE = 0.15

# all small constants ship as two packed tensors (one bf16, one f32 that also
# carries seg/wgt) so startup is 2 big DMAs instead of ~24 serialized HWDGE ops
WPACK_BF = ([("iota", 128, 128), ("W1l", 128, 512), ("W1r", 128, 512)]
            + [(f"W2l{k}", 128, 256) for k in range(4)]
            + [(f"W2r{k}", 128, 256) for k in range(4)]
            + [(f"W3l{k}", 128, 64) for k in range(2)]
            + [(f"W3r{k}", 128, 64) for k in range(2)]
            + [("Wp", 64, 32), ("Wf1", 32, 32), ("Wf2", 32, 2),
               ("ones", 1, 512), ("bf2r", 1, 2)])
WPACK_F32 = [("b1c", 128, 4), ("b2c", 128, 2), ("b3c", 64, 1),
             ("bpc", 32, 1), ("bf1c", 32, 1)]


def pack_offsets(spec):
    offs, o = {}, 0
    for name, part, cols in spec:
        offs[name] = (o, part, cols)
        o += cols
    return offs, o


BF_OFFS, BF_COLS = pack_offsets(WPACK_BF)
F32_OFFS, F32_COLS = pack_offsets(WPACK_F32)


def ceil_div(a, b):
    return (a + b - 1) // b


def build_kernel(cfg, sched, timing_mode=False):
    """cfg: dict(n_cores, n_nodes, slice, slice_pad, sb_sizes, ...)
    sched: from build_structure (list of superblock dicts)."""
    C = cfg["n_cores"]
    SP = cfg["slice_pad"]
    HI_BASE = cfg["hi_base"]
    N_LO_SB = cfg["n_lo_sb"]
    PIDN = cfg["pid_n"]
    NTT = sum(len(s["tiles"]) for s in sched)      # total tiles
    NID = NTT * W                                   # total edge slots
    MAXSBC = max(s["sb_sz"] for s in sched) * W
    MAXNT = max(t1 - t0 for s in sched for (_, t0, t1, _) in s["chunks"])
    sb_rows = [C * s["sb_sz"] * W for s in sched]   # pid rows per superblock
    sb_row0 = np.cumsum([0] + sb_rows).tolist()
    HI_ROWS = PIDN - HI_BASE

    def sb_chunks(sbc):
        return [(i * 512, min(512, sbc - i * 512)) for i in range(ceil_div(sbc, 512))]

    nc = bacc.Bacc("TRN2", target_bir_lowering=False, debug=False,
                   enable_asserts=True, num_devices=(1 if timing_mode else C))

    # ---------------- DRAM I/O ----------------
    xpid_d = nc.dram_tensor("xpid", [PIDN, 128], BF16, kind="ExternalInput")
    xT_d = nc.dram_tensor("xT", [128, SP], BF16, kind="ExternalInput")
    idx_d = nc.dram_tensor("idx", [128, NID // 16], mybir.dt.int16, kind="ExternalInput")
    wb_d = nc.dram_tensor("wb", [128, BF_COLS], BF16, kind="ExternalInput")
    wf_d = nc.dram_tensor("wf", [128, 2 * NTT + F32_COLS], F32, kind="ExternalInput")
    out_d = nc.dram_tensor("out", [SP, 2], F32, kind="ExternalOutput")

    with tile.TileContext(nc) as tc:
        with (
            tc.tile_pool(name="const", bufs=1) as cp,
            tc.tile_pool(name="big", bufs=1) as bp,       # long-lived buffers
            tc.tile_pool(name="gath", bufs=6) as gp,      # gather chunks
            tc.tile_pool(name="work", bufs=16) as wp,      # onehot / drains / stages
            tc.tile_pool(name="psA", bufs=1, space="PSUM") as psA,   # agg psums
            tc.tile_pool(name="psB", bufs=2, space="PSUM") as psB,   # dense psums
            tc.tile_pool(name="dram", bufs=1, space="DRAM") as dp,
        ):
            # ---------------- constants ----------------
            wf_t = cp.tile([128, 2 * NTT + F32_COLS], F32, name="wf_t")
            nc.sync.dma_start(wf_t[:], wf_d.ap())
            wb_t = cp.tile([128, BF_COLS], BF16, name="wb_t")
            nc.sync.dma_start(wb_t[:], wb_d.ap())

            def w(name, c0=None, c1=None):
                o, p, c = BF_OFFS[name]
                if c0 is None:
                    c0, c1 = 0, c
                return wb_t[0:p, o + c0:o + c1]

            def wf(name, c0=None, c1=None):
                o, p, c = F32_OFFS[name]
                if c0 is None:
                    c0, c1 = 0, c
                return wf_t[0:p, 2 * NTT + o + c0:2 * NTT + o + c1]

            # persistent self terms (SBUF-resident)
            selfb2 = [cp.tile([128, SP], BF16, name=f"selfb2_{m}") for m in range(2)]
            selfb3 = cp.tile([64, SP], BF16)

            # DRAM scratch: allgathered tables, split lo/hi so gathers from
            # the lo pids only depend on the lo superblocks' collectives
            t2_lo = dp.tile([HI_BASE, 256], BF16)
            t2_hi = dp.tile([HI_ROWS, 256], BF16)
            t3_lo = dp.tile([HI_BASE, 128], BF16)
            t3_hi = dp.tile([HI_ROWS, 128], BF16)
            t2_slice = dp.tile([SP, 256], BF16)
            t3_slice = dp.tile([SP, 128], BF16)

            def emit_ag(si, slice_t, lo_t, hi_t):
                if timing_mode:
                    return
                col0 = sched[si]["blocks"][0] * W
                sbc = sched[si]["sb_sz"] * W
                r0 = sb_row0[si]
                dst_t, doff = (lo_t, 0) if si < N_LO_SB else (hi_t, HI_BASE)
                nc.gpsimd.collective_compute(
                    "AllGather", ALU.bypass, replica_groups=[list(range(C))],
                    ins=[slice_t[col0:col0 + sbc, :].opt()],
                    outs=[dst_t[r0 - doff:r0 - doff + C * sbc, :].opt()])

            # prime the gather-pool buffers so slots skipped by trimmed
            # num_idxs never expose uninitialized SBUF (NaN x 0 = NaN in PE)
            for pi in range(6):
                gz = gp.tile([128, MAXNT * 256], BF16, name=f"gz{pi}", tag="g")
                nc.vector.memset(gz[:], 0.0)

            def leaky(dst_ap, src_ap, bias=0.0):
                # Prelu honors alpha on HW (Lrelu ignores it: fixed slope)
                nc.scalar.activation(dst_ap, src_ap, AF.Prelu, bias=bias,
                                     alpha=SLOPE)

            # =========== generic aggregation phase ===========
            def agg_phase(d, row_elems, lo_ap, hi_ap, idx_t, drain_fn,
                          after_sb, tag):
                """d: used dims; row_elems: table row width.
                drain_fn(si, mp, ps_ap, col0, ncols): consume psum [dmp, ncols]
                holding agg_T rows [mp*128, ...) cols [col0, col0+ncols).
                after_sb(si, col0, sbc): emit dependent work for the sb."""
                n_mp = ceil_div(d, 128)
                dmp = min(d, 128)
                toff = 0
                for si, s in enumerate(sched):
                    sbc = s["sb_sz"] * W
                    tiles = s["tiles"]
                    bank_of = lambda bl: (bl * W) // 512
                    first, last = {}, {}
                    for i, (h, bl) in enumerate(tiles):
                        bk = bank_of(bl)
                        first.setdefault(bk, i)
                        last[bk] = i
                    ps = [psA.tile([dmp, sbc], F32, name=f"ps{tag}{mp}",
                                   tag=f"aggps{mp}", bufs=2)
                          for mp in range(n_mp)]
                    for (h, t0, t1, nidx) in s["chunks"]:
                        nt = t1 - t0
                        g = gp.tile([128, nt, row_elems], BF16, name=f"g{tag}", tag="g")
                        base = lo_ap if h == 0 else hi_ap
                        nc.gpsimd.dma_gather(
                            g[:, :nt, :], base,
                            idx_t[:, (toff + t0) * 8:(toff + t1) * 8],
                            nidx, nidx, elem_size=row_elems, single_packet=False)
                        for t in range(t0, t1):
                            h_, bl = tiles[t]
                            bk = bank_of(bl)
                            oh = wp.tile([128, W], BF16, name=f"oh{tag}", tag="oh", bufs=16)
                            gt = toff + t
                            nc.vector.tensor_scalar(
                                oh[:], w("iota"), wf_t[:, gt:gt + 1],
                                wf_t[:, NTT + gt:NTT + gt + 1],
                                ALU.is_equal, ALU.mult)
                            for mp in range(n_mp):
                                nc.tensor.matmul(
                                    ps[mp][:, bl * W:(bl + 1) * W],
                                    g[:, t - t0, mp * dmp:(mp + 1) * dmp],
                                    oh[:],
                                    start=(first[bk] == t), stop=(last[bk] == t))
                    col0 = s["blocks"][0] * W
                    # drain the single-buffered mps first: the next
                    # superblock's matmuls wait on their psum buffers
                    for mp in reversed(range(n_mp)):
                        cc = 0
                        while cc < sbc:
                            n = min(512, sbc - cc)
                            drain_fn(si, mp, ps[mp][:, cc:cc + n], col0 + cc, n)
                            cc += n
                    if after_sb is not None:
                        after_sb(si, col0, sbc)
                    toff += len(tiles)

            # =========== L1: agg + interleaved dense ===========
            # only the first superblock's indices load upfront; the rest (and
            # most of xT) load behind the first superblock's gathers so the
            # DMA engines start useful work immediately
            idx_t = bp.tile([128, NID // 16], mybir.dt.int16, name="idx_t", tag="big1")
            NT0 = len(sched[0]["tiles"])
            SBC0 = sched[0]["sb_sz"] * W
            nc.sync.dma_start(idx_t[:, :NT0 * 8], idx_d.ap()[:, :NT0 * 8])
            xT_t = bp.tile([128, SP], BF16, name="xT_t", tag="big3")
            nc.sync.dma_start(xT_t[:, :SBC0], xT_d.ap()[:, :SBC0])

            agg1_sb = {}

            def drain1(si, mp, ps_ap, c0, n):
                if si not in agg1_sb:
                    agg1_sb[si] = (bp.tile([128, MAXSBC], BF16, name=f"agg1_{si}",
                                           tag="agg1", bufs=3),
                                   sched[si]["blocks"][0] * W)
                t_, col0 = agg1_sb[si]
                nc.scalar.activation(t_[:, c0 - col0:c0 - col0 + n], ps_ap, AF.Copy)

            def l1_dense(si, col0, sbc):
                if si == 0:
                    nc.sync.dma_start(idx_t[:, NT0 * 8:], idx_d.ap()[:, NT0 * 8:])
                    nc.sync.dma_start(xT_t[:, SBC0:], xT_d.ap()[:, SBC0:])
                a1, _ = agg1_sb.pop(si)
                for (off, n) in sb_chunks(sbc):
                    c0 = col0 + off
                    h1 = [wp.tile([128, 512], BF16, name=f"h1_{m}", tag=f"h1_{m}",
                                  bufs=2) for m in range(4)]
                    for m in range(4):
                        ph = psB.tile([128, 512], F32, name="ph1", tag="dense")
                        nc.tensor.matmul(ph[:, :n], w("W1l", m * 128, (m + 1) * 128),
                                         a1[:, off:off + n], start=True, stop=False)
                        nc.tensor.matmul(ph[:, :n], w("W1r", m * 128, (m + 1) * 128),
                                         xT_t[:, c0:c0 + n], start=False, stop=True)
                        leaky(h1[m][:, :n], ph[:, :n], bias=wf("b1c", m, m + 1))
                    for sub in range(n // 128):
                        pt = psB.tile([128, 256], F32, name="pt2", tag="dense")
                        for k in range(4):
                            nc.tensor.matmul(pt[:], h1[k][:, sub * 128:(sub + 1) * 128],
                                             w(f"W2l{k}"),
                                             start=(k == 0), stop=(k == 3))
                        st = wp.tile([128, 256], BF16, name="st2", tag="st2")
                        nc.scalar.activation(st[:], pt[:], AF.Copy)
                        r0 = c0 + sub * 128
                        nc.sync.dma_start(t2_slice[r0:r0 + 128, :], st[:])
                    for m in range(2):
                        pss = psB.tile([128, 512], F32, name="ps2", tag="dense")
                        for k in range(4):
                            nc.tensor.matmul(pss[:, :n],
                                             w(f"W2r{k}", m * 128, (m + 1) * 128),
                                             h1[k][:, :n], start=(k == 0), stop=(k == 3))
                        nc.scalar.activation(selfb2[m][:, c0:c0 + n], pss[:, :n],
                                             AF.Identity, bias=wf("b2c", m, m + 1))
                emit_ag(si, t2_slice, t2_lo, t2_hi)

            agg_phase(128, 128, xpid_d.ap(), xpid_d.ap()[HI_BASE:, :],
                      idx_t, drain1, l1_dense, "1")

            # =========== L2 ===========
            h2_sb = {}

            def drain2(si, mp, ps_ap, c0, n):
                key = (si, mp)
                if key not in h2_sb:
                    h2_sb[key] = (bp.tile([128, MAXSBC], BF16, name=f"h2_{si}_{mp}",
                                          tag=f"h2m{mp}", bufs=2),
                                  sched[si]["blocks"][0] * W)
                t_, col0 = h2_sb[key]
                pre = wp.tile([128, 512], BF16, name="pre2", tag="pre")
                nc.vector.tensor_tensor(pre[:, :n], ps_ap,
                                        selfb2[mp][:, c0:c0 + n], ALU.add)
                leaky(t_[:, c0 - col0:c0 - col0 + n], pre[:, :n])

            def l2_dense(si, col0, sbc):
                h2m = [h2_sb.pop((si, mp))[0] for mp in range(2)]
                for sub in range(sbc // 128):
                    pt = psB.tile([128, 64], F32, name="pt3", tag="dense")
                    s0 = col0 + sub * 128
                    for k in range(2):
                        nc.tensor.matmul(pt[:], h2m[k][:, sub * 128:(sub + 1) * 128],
                                         w(f"W3l{k}"),
                                         start=(k == 0), stop=(k == 1))
                    st = wp.tile([128, 64], BF16, name="st3", tag="st3")
                    nc.scalar.activation(st[:], pt[:], AF.Copy)
                    nc.sync.dma_start(t3_slice[s0:s0 + 128, :64], st[:])
                for (off, n) in sb_chunks(sbc):
                    c0 = col0 + off
                    pss = psB.tile([64, 512], F32, name="ps3", tag="dense")
                    for k in range(2):
                        nc.tensor.matmul(pss[:, :n], w(f"W3r{k}"),
                                         h2m[k][:, off:off + n],
                                         start=(k == 0), stop=(k == 1))
                    nc.scalar.activation(selfb3[:, c0:c0 + n], pss[:, :n],
                                         AF.Identity, bias=wf("b3c"))
                emit_ag(si, t3_slice, t3_lo, t3_hi)

            agg_phase(256, 256, t2_lo[:], t2_hi[:], idx_t,
                      drain2, l2_dense, "2")

            # =========== L3 + head ===========
            h3_sb = {}

            def drain3(si, mp, ps_ap, c0, n):
                if si not in h3_sb:
                    h3_sb[si] = (bp.tile([64, MAXSBC], BF16, name=f"h3_{si}",
                                         tag="h3", bufs=2),
                                 sched[si]["blocks"][0] * W)
                t_, col0 = h3_sb[si]
                pre = wp.tile([64, 512], BF16, name="pre3", tag="pre")
                nc.vector.tensor_tensor(pre[:, :n], ps_ap,
                                        selfb3[:, c0:c0 + n], ALU.add)
                leaky(t_[:, c0 - col0:c0 - col0 + n], pre[:, :n])

            def head(si, col0, sbc):
                h3t, _ = h3_sb.pop(si)
                for (off, n) in sb_chunks(sbc):
                    c0 = col0 + off
                    pp = psB.tile([32, 512], F32, name="pp", tag="dense")
                    nc.tensor.matmul(pp[:, :n], w("Wp"), h3t[:, off:off + n],
                                     start=True, stop=True)
                    p_t = wp.tile([32, 512], BF16, name="p", tag="p", bufs=2)
                    nc.scalar.activation(p_t[:, :n], pp[:, :n], AF.Identity,
                                         bias=wf("bpc"))

                    pf = psB.tile([32, 512], F32, name="pf", tag="dense")
                    nc.tensor.matmul(pf[:, :n], w("Wf1"), p_t[:, :n],
                                     start=True, stop=True)
                    f1 = wp.tile([32, 512], BF16, name="f1", tag="f1", bufs=2)
                    leaky(f1[:, :n], pf[:, :n], bias=wf("bf1c"))

                    for sub in range(n // 128):
                        po = psB.tile([128, 2], F32, name="po", tag="dense")
                        nc.tensor.matmul(po[:], f1[:, sub * 128:(sub + 1) * 128],
                                         w("Wf2"), start=True, stop=False)
                        nc.tensor.matmul(po[:], w("ones", 0, 128), w("bf2r"),
                                         start=False, stop=True)
                        ot = wp.tile([128, 2], F32, name="ot", tag="ot")
                        nc.scalar.activation(ot[:], po[:], AF.Copy)
                        s0 = c0 + sub * 128
                        nc.sync.dma_start(out_d.ap()[s0:s0 + 128, :], ot[:])

            agg_phase(64, 128, t3_lo[:], t3_hi[:], idx_t,
                      drain3, head, "3")

    nc.compile()
    return nc


def make_core_inputs(cfg, sched, percore, inp):
    """Build per-core in_maps from the problem inputs dict."""
    C = cfg["n_cores"]
    SP = cfg["slice_pad"]
    SL = cfg["slice"]
    NN = cfg["n_nodes"]
    BFNP = mybir.dt.np(BF16)
    x = np.asarray(inp["x"], np.float32)
    iota = np.tile(np.arange(W, dtype=np.float32), (128, 1))

    def bf(a):
        return np.ascontiguousarray(np.asarray(a, np.float32)).astype(BFNP)

    xpid = np.zeros((cfg["pid_n"], 128), np.float32)
    xpid[pid_of(np.arange(NN), cfg)] = x

    # packed bf16 constants
    wvals_bf = dict(
        iota=iota,
        W1l=np.asarray(inp["W1l"], np.float32),
        W1r=np.asarray(inp["W1r"], np.float32),
        Wp=np.asarray(inp["Wp"], np.float32),
        Wf1=np.asarray(inp["Wf1"], np.float32),
        Wf2=np.asarray(inp["Wf2"], np.float32),
        ones=np.ones((1, 512), np.float32),
        bf2r=np.asarray(inp["bf2"], np.float32).reshape(1, -1),
    )
    for k in range(4):
        wvals_bf[f"W2l{k}"] = np.asarray(inp["W2l"], np.float32)[k * 128:(k + 1) * 128]
        wvals_bf[f"W2r{k}"] = np.asarray(inp["W2r"], np.float32)[k * 128:(k + 1) * 128]
    for k in range(2):
        wvals_bf[f"W3l{k}"] = np.asarray(inp["W3l"], np.float32)[k * 128:(k + 1) * 128]
        wvals_bf[f"W3r{k}"] = np.asarray(inp["W3r"], np.float32)[k * 128:(k + 1) * 128]
    wb = np.zeros((128, BF_COLS), np.float32)
    for name, (o, p, cc) in BF_OFFS.items():
        wb[0:p, o:o + cc] = wvals_bf[name]

    wvals_f32 = dict(
        b1c=np.asarray(inp["b1"], np.float32).reshape(4, 128).T,
        b2c=np.asarray(inp["b2"], np.float32).reshape(2, 128).T,
        b3c=np.asarray(inp["b3"], np.float32).reshape(64, 1),
        bpc=np.asarray(inp["bp"], np.float32).reshape(32, 1),
        bf1c=np.asarray(inp["bf1"], np.float32).reshape(32, 1),
    )

    shared = dict(wb=wb.astype(BFNP), xpid=xpid.astype(BFNP))
    in_maps = []
    for c in range(C):
        xs = np.zeros((128, SP), np.float32)
        xs[:, :SL] = x[c * SL:(c + 1) * SL].T
        ntt = len(percore[c]["seg"]) // W
        wfp = np.zeros((128, 2 * ntt + F32_COLS), np.float32)
        wfp[:, :ntt] = pack_tilewise(percore[c]["seg"], ntt)
        wfp[:, ntt:2 * ntt] = pack_tilewise(percore[c]["wgt"], ntt)
        for name, (o, p, cc) in F32_OFFS.items():
            wfp[0:p, 2 * ntt + o:2 * ntt + o + cc] = wvals_f32[name]
        m = dict(shared)
        m.update(
            xT=xs.astype(BFNP),
            idx=pack_gather_idx(percore[c]["idx"]),
            wf=wfp,
        )
        in_maps.append(m)
    return in_maps


# ----------------------------------------------------------------------
# public entry point
# ----------------------------------------------------------------------
_CACHE = {}


def _get_compiled(edge_index):
    key = hash(edge_index.tobytes())
    if key not in _CACHE:
        cfg = CFG_FULL
        sched, percore = build_structure(edge_index, cfg)
        nc = build_kernel(cfg, sched)
        _CACHE[key] = (cfg, sched, percore, nc)
    return _CACHE[key]


def _run(inputs, trace=False):
    inputs = {k: np.asarray(v) for k, v in inputs.items()}
    edge_index = np.asarray(inputs["edge_index"], np.int32)
    cfg, sched, percore, nc = _get_compiled(edge_index)
    in_maps = make_core_inputs(cfg, sched, percore, inputs)
    res = run_bass_kernel_spmd(nc, in_maps, core_ids=list(range(cfg["n_cores"])),
                               trace=trace)
    out = np.concatenate([res.results[c]["out"][:cfg["slice"]]
                          for c in range(cfg["n_cores"])]).astype(np.float32)
    return out, res


def kernel(**inputs):
    out, _ = _run(inputs)
    return out
